# revision 18
# baseline (speedup 1.0000x reference)
"""ChebyshevKAN layer on 8 Trainium2 NeuronCores.

y[b,o] = sum_{i,j} T_j(xn[b,i]) * C[i,o,j],  xn = per-row min/max normalize to [-1,1]

Strategy (per core, batch-sharded 8 ways => 1024 rows/core):
  - normalize x rows on ACT (scale/bias per partition), cast fp16
  - DMA-transpose xn into [i, b] layout tiles
  - Chebyshev recurrence T_{j+1} = 2 xn T_j - T_{j-1} on DVE (fp16, fused
    scalar_tensor_tensor), all T_j tiles cached in SBUF (16 MB)
  - coeffs pre-transposed on host to [j, i, o] fp16; streamed twice (once per
    512-wide output tile); 1152 accumulating fp16 matmuls into 8 PSUM banks
"""

import sys

sys.path.insert(0, "/opt/trn_rl_repo")

import numpy as np

import concourse.bass as bass  # noqa: F401  (bass must import before tile)
import concourse.tile as tile
from concourse import bacc, mybir
from concourse.bass_utils import run_bass_kernel_spmd

NCORES = 8
B_FULL = 8192
B_SH = B_FULL // NCORES  # 1024 rows per core
I_DIM = 1024
O_DIM = 1024
NJ = 9  # degrees 0..8
P = 128
NBT = B_SH // P  # 8 batch tiles per core
NIC = I_DIM // P  # 8 contraction chunks
OT = 512  # output tile width
NOT = O_DIM // OT  # 2

_PROGRAM_CACHE = {}


def build_program(repeat=1):
    """Build + compile the per-core Bass program (cached).

    repeat>1 wraps the whole body in an on-device loop — used only for
    timing (amortizes host dispatch overhead over `repeat` kernel runs).
    """
    if repeat in _PROGRAM_CACHE:
        return _PROGRAM_CACHE[repeat]

    f16 = mybir.dt.float16
    f32 = mybir.dt.float32

    nc = bacc.Bacc("TRN2", target_bir_lowering=False, debug=False, num_devices=NCORES)
    xs_ext = nc.dram_tensor("xs", [B_SH, I_DIM], f32, kind="ExternalInput")
    cj_ext = nc.dram_tensor("cj", [NJ, I_DIM, O_DIM], f16, kind="ExternalInput")
    y_ext = nc.dram_tensor("y", [B_SH, O_DIM], f32, kind="ExternalOutput")

    import contextlib

    with tile.TileContext(nc) as tc:
        with (
            tc.tile_pool(name="tall", bufs=1) as tp,
            tc.tile_pool(name="xp", bufs=2) as xpool,
            tc.tile_pool(name="sm", bufs=16) as spool,
            tc.tile_pool(name="cp", bufs=6) as cpool,
            tc.tile_pool(name="op", bufs=4) as opool,
            tc.tile_pool(name="ps", bufs=8, space="PSUM") as pspool,
            tc.For_i(0, repeat, 1) if repeat > 1 else contextlib.nullcontext(),
        ):
            # T_all[:, j-1, ic, bt, :] holds T_j in transposed [i, b] layout
            T_all = tp.tile([P, NJ - 1, NIC, NBT, P], f16)
            ones1 = tp.tile([P, 1], f16)
            nc.vector.memset(ones1, 1.0)

            # ---- Phase A: normalize + transpose, per batch tile ----
            for bt in range(NBT):
                x_sb = xpool.tile([P, I_DIM], f32)
                nc.sync.dma_start(out=x_sb, in_=xs_ext[bt * P : (bt + 1) * P, :])
                mx = spool.tile([P, 1], f32)
                mn = spool.tile([P, 1], f32)
                nc.vector.tensor_reduce(
                    out=mx, in_=x_sb, op=mybir.AluOpType.max, axis=mybir.AxisListType.X
                )
                nc.vector.tensor_reduce(
                    out=mn, in_=x_sb, op=mybir.AluOpType.min, axis=mybir.AxisListType.X
                )
                st2 = spool.tile([P, 2], f32)
                s = st2[:, 0:1]
                t = st2[:, 1:2]
                rng = spool.tile([P, 1], f32)
                nc.vector.tensor_sub(out=rng, in0=mx, in1=mn)
                nc.vector.reciprocal(out=s, in_=rng)
                nc.vector.tensor_scalar_mul(s, s, 2.0)
                # t = (mn * -1) * s - 1
                nc.vector.scalar_tensor_tensor(
                    out=t, in0=mn, scalar=-1.0, in1=s,
                    op0=mybir.AluOpType.mult, op1=mybir.AluOpType.mult,
                )
                nc.vector.tensor_scalar_add(t, t, -1.0)

                xt16 = xpool.tile([P, I_DIM], f16)
                nc.scalar.activation(
                    out=xt16, in_=x_sb,
                    func=mybir.ActivationFunctionType.Identity,
                    bias=t, scale=s,
                )
                # T_1 = xn, transposed into [i, b] tiles
                nc.sync.dma_start_transpose(out=T_all[:, 0, :, bt, :], in_=xt16)

            # ---- Phase B: Chebyshev recurrence (j-major so PE can follow) ----
            for j in range(2, NJ):
                for bt in range(NBT):
                    t1 = T_all[:, 0, :, bt, :]
                    tprev = T_all[:, j - 2, :, bt, :]
                    cur = T_all[:, j - 1, :, bt, :]
                    prod = xpool.tile([P, NIC, P], f16)
                    nc.vector.scalar_tensor_tensor(
                        out=prod, in0=tprev, scalar=2.0, in1=t1,
                        op0=mybir.AluOpType.mult, op1=mybir.AluOpType.mult,
                    )
                    if j == 2:
                        nc.vector.tensor_scalar_add(cur, prod, -1.0)
                    else:
                        nc.vector.tensor_sub(
                            out=cur, in0=prod, in1=T_all[:, j - 3, :, bt, :]
                        )

            # ---- Phase C: matmuls, coeffs streamed once per output tile ----
            # T_0 = 1, so its contribution is the per-column bias
            # bias[o] = sum_i C[i,o,0]: 8 tiny M=1 matmuls per output tile,
            # then seeded into each accumulator by a K=1 rank-1 matmul --
            # replaces 128 full-size degree-0 matmuls.
            ones_row = tp.tile([1, P], f16)
            nc.vector.memset(ones_row, 1.0)
            bias16s = []
            for ot in range(NOT):
                ps_bias = pspool.tile([P, OT], f32, name=f"psb{ot}", tag="psacc")
                for ic in range(NIC):
                    c0_sb = cpool.tile([P, OT], f16, name=f"c0_{ot}_{ic}", tag="c_sb")
                    nc.sync.dma_start(
                        out=c0_sb,
                        in_=cj_ext[0, ic * P : (ic + 1) * P, ot * OT : (ot + 1) * OT],
                    )
                    nc.tensor.matmul(
                        ps_bias[0:1, :], lhsT=ones1, rhs=c0_sb,
                        start=(ic == 0), stop=(ic == NIC - 1),
                    )
                bias16 = tp.tile([1, OT], f16, name=f"bias16_{ot}", tag=f"bias16_{ot}")
                nc.scalar.copy(out=bias16, in_=ps_bias[0:1, :])
                bias16s.append(bias16)

            for ot in range(NOT):
                psums = [
                    pspool.tile([P, OT], f32, name=f"ps{ot}_{bt}", tag="psacc")
                    for bt in range(NBT)
                ]
                # seed accumulators with the T_0 bias via K=1 rank-1 matmul
                for bt in range(NBT):
                    nc.tensor.matmul(
                        psums[bt], lhsT=ones_row, rhs=bias16s[ot],
                        start=True, stop=False,
                    )
                for j in range(1, NJ):
                    for ic in range(NIC):
                        c_sb = cpool.tile([P, OT], f16, name=f"c_{ot}_{j}_{ic}", tag="c_sb")
                        nc.sync.dma_start(
                            out=c_sb,
                            in_=cj_ext[j, ic * P : (ic + 1) * P, ot * OT : (ot + 1) * OT],
                        )
                        for bt in range(NBT):
                            nc.tensor.matmul(
                                psums[bt], lhsT=T_all[:, j - 1, ic, bt, :], rhs=c_sb,
                                start=False,
                                stop=(j == NJ - 1 and ic == NIC - 1),
                            )
                for bt in range(NBT):
                    o_sb = opool.tile([P, OT], f32)
                    nc.scalar.copy(out=o_sb, in_=psums[bt])
                    nc.sync.dma_start(
                        out=y_ext[bt * P : (bt + 1) * P, ot * OT : (ot + 1) * OT],
                        in_=o_sb,
                    )

    nc.compile()
    _PROGRAM_CACHE[repeat] = nc
    return nc


def host_prep(x, cheby_coeffs):
    """Host-side layout prep: coeffs -> [j, i, o] fp16, x row shards."""
    cj = np.ascontiguousarray(np.transpose(np.asarray(cheby_coeffs), (2, 0, 1))).astype(
        np.float16
    )
    x = np.asarray(x, dtype=np.float32).reshape(B_FULL, I_DIM)
    in_maps = [
        {"xs": x[i * B_SH : (i + 1) * B_SH], "cj": cj} for i in range(NCORES)
    ]
    return in_maps


def kernel(x, cheby_coeffs):
    nc = build_program(1)
    in_maps = host_prep(x, cheby_coeffs)
    res = run_bass_kernel_spmd(nc, in_maps, list(range(NCORES)))
    return np.concatenate([r["y"] for r in res.results], axis=0)


# revision 20
# speedup vs baseline: 1.0133x; 1.0133x over previous
"""ChebyshevKAN layer on 8 Trainium2 NeuronCores.

y[b,o] = sum_{i,j} T_j(xn[b,i]) * C[i,o,j],  xn = per-row min/max normalize to [-1,1]

Strategy (per core, batch-sharded 8 ways => 1024 rows/core):
  - normalize x rows on ACT (scale/bias per partition), cast fp16
  - DMA-transpose xn into [i, b] layout tiles
  - Chebyshev recurrence T_{j+1} = 2 xn T_j - T_{j-1} on DVE (fp16, fused
    scalar_tensor_tensor), all T_j tiles cached in SBUF (16 MB)
  - coeffs pre-transposed on host to [j, i, o] fp16; streamed twice (once per
    512-wide output tile); 1152 accumulating fp16 matmuls into 8 PSUM banks
"""

import sys

sys.path.insert(0, "/opt/trn_rl_repo")

import numpy as np

import concourse.bass as bass  # noqa: F401  (bass must import before tile)
import concourse.tile as tile
from concourse import bacc, mybir
from concourse.bass_utils import run_bass_kernel_spmd

NCORES = 8
B_FULL = 8192
B_SH = B_FULL // NCORES  # 1024 rows per core
I_DIM = 1024
O_DIM = 1024
NJ = 9  # degrees 0..8
P = 128
NBT = B_SH // P  # 8 batch tiles per core
NIC = I_DIM // P  # 8 contraction chunks
OT = 512  # output tile width
NOT = O_DIM // OT  # 2

_PROGRAM_CACHE = {}


def build_program(repeat=1):
    """Build + compile the per-core Bass program (cached).

    repeat>1 wraps the whole body in an on-device loop — used only for
    timing (amortizes host dispatch overhead over `repeat` kernel runs).
    """
    if repeat in _PROGRAM_CACHE:
        return _PROGRAM_CACHE[repeat]

    f16 = mybir.dt.float16
    f32 = mybir.dt.float32

    nc = bacc.Bacc("TRN2", target_bir_lowering=False, debug=False, num_devices=NCORES)
    xs_ext = nc.dram_tensor("xs", [B_SH, I_DIM], f32, kind="ExternalInput")
    cj_ext = nc.dram_tensor("cj", [NJ, I_DIM, O_DIM], f16, kind="ExternalInput")
    y_ext = nc.dram_tensor("y", [B_SH, O_DIM], f32, kind="ExternalOutput")

    import contextlib

    with tile.TileContext(nc) as tc:
        with (
            tc.tile_pool(name="tall", bufs=1) as tp,
            tc.tile_pool(name="xp", bufs=2) as xpool,
            tc.tile_pool(name="sm", bufs=16) as spool,
            tc.tile_pool(name="cp", bufs=6) as cpool,
            tc.tile_pool(name="op", bufs=4) as opool,
            tc.tile_pool(name="ps", bufs=8, space="PSUM") as pspool,
            tc.For_i(0, repeat, 1) if repeat > 1 else contextlib.nullcontext(),
        ):
            # T_all[:, j-1, ic, bt, :] holds T_j in transposed [i, b] layout
            T_all = tp.tile([P, NJ - 1, NIC, NBT, P], f16)
            ones1 = tp.tile([P, 1], f16)
            nc.vector.memset(ones1, 1.0)

            # ---- Phase A: normalize + transpose, per batch tile ----
            for bt in range(NBT):
                x_sb = xpool.tile([P, I_DIM], f32)
                nc.sync.dma_start(out=x_sb, in_=xs_ext[bt * P : (bt + 1) * P, :])
                mx = spool.tile([P, 1], f32)
                mn = spool.tile([P, 1], f32)
                nc.vector.tensor_reduce(
                    out=mx, in_=x_sb, op=mybir.AluOpType.max, axis=mybir.AxisListType.X
                )
                nc.vector.tensor_reduce(
                    out=mn, in_=x_sb, op=mybir.AluOpType.min, axis=mybir.AxisListType.X
                )
                st2 = spool.tile([P, 2], f32)
                s = st2[:, 0:1]
                t = st2[:, 1:2]
                rng = spool.tile([P, 1], f32)
                nc.vector.tensor_sub(out=rng, in0=mx, in1=mn)
                nc.vector.reciprocal(out=s, in_=rng)
                nc.vector.tensor_scalar_mul(s, s, 2.0)
                # t = (mn * -1) * s - 1
                nc.vector.scalar_tensor_tensor(
                    out=t, in0=mn, scalar=-1.0, in1=s,
                    op0=mybir.AluOpType.mult, op1=mybir.AluOpType.mult,
                )
                nc.vector.tensor_scalar_add(t, t, -1.0)

                xt16 = xpool.tile([P, I_DIM], f16)
                nc.scalar.activation(
                    out=xt16, in_=x_sb,
                    func=mybir.ActivationFunctionType.Identity,
                    bias=t, scale=s,
                )
                # T_1 = xn, transposed into [i, b] tiles
                nc.sync.dma_start_transpose(out=T_all[:, 0, :, bt, :], in_=xt16)

            # ---- Phase B: Chebyshev recurrence (j-major so PE can follow) ----
            for j in range(2, NJ):
                for bt in range(NBT):
                    t1 = T_all[:, 0, :, bt, :]
                    tprev = T_all[:, j - 2, :, bt, :]
                    cur = T_all[:, j - 1, :, bt, :]
                    prod = xpool.tile([P, NIC, P], f16)
                    nc.vector.scalar_tensor_tensor(
                        out=prod, in0=tprev, scalar=2.0, in1=t1,
                        op0=mybir.AluOpType.mult, op1=mybir.AluOpType.mult,
                    )
                    if j == 2:
                        nc.vector.tensor_scalar_add(cur, prod, -1.0)
                    else:
                        nc.vector.tensor_sub(
                            out=cur, in0=prod, in1=T_all[:, j - 3, :, bt, :]
                        )

            # ---- Phase C: matmuls, coeffs streamed once per output tile ----
            # T_0 = 1, so its contribution is the per-column bias
            # bias[o] = sum_i C[i,o,0]: 8 tiny M=1 matmuls per output tile,
            # then seeded into each accumulator by a K=1 rank-1 matmul --
            # replaces 128 full-size degree-0 matmuls.
            ones_row = tp.tile([1, P], f16)
            nc.vector.memset(ones_row, 1.0)
            bias16s = []
            for ot in range(NOT):
                ps_bias = pspool.tile([P, OT], f32, name=f"psb{ot}", tag="psacc")
                for ic in range(NIC):
                    c0_sb = cpool.tile([P, OT], f16, name=f"c0_{ot}_{ic}", tag="c_sb")
                    nc.sync.dma_start(
                        out=c0_sb,
                        in_=cj_ext[0, ic * P : (ic + 1) * P, ot * OT : (ot + 1) * OT],
                    )
                    nc.tensor.matmul(
                        ps_bias[0:1, :], lhsT=ones1, rhs=c0_sb,
                        start=(ic == 0), stop=(ic == NIC - 1),
                    )
                bias16 = tp.tile([1, OT], f16, name=f"bias16_{ot}", tag=f"bias16_{ot}")
                nc.scalar.copy(out=bias16, in_=ps_bias[0:1, :])
                bias16s.append(bias16)

            for ot in range(NOT):
                psums = [
                    pspool.tile([P, OT], f32, name=f"ps{ot}_{bt}", tag="psacc")
                    for bt in range(NBT)
                ]
                # seed accumulators with the T_0 bias via K=1 rank-1 matmul
                for bt in range(NBT):
                    nc.tensor.matmul(
                        psums[bt], lhsT=ones_row, rhs=bias16s[ot],
                        start=True, stop=False,
                    )
                for j in range(1, NJ):
                    for ic in range(NIC):
                        c_sb = cpool.tile([P, OT], f16, name=f"c_{ot}_{j}_{ic}", tag="c_sb")
                        nc.sync.dma_start(
                            out=c_sb,
                            in_=cj_ext[j, ic * P : (ic + 1) * P, ot * OT : (ot + 1) * OT],
                        )
                        for bt in range(NBT):
                            nc.tensor.matmul(
                                psums[bt], lhsT=T_all[:, j - 1, ic, bt, :], rhs=c_sb,
                                start=False,
                                stop=(j == NJ - 1 and ic == NIC - 1),
                            )
                for bt in range(NBT):
                    o_sb = opool.tile([P, OT], f32)
                    nc.scalar.copy(out=o_sb, in_=psums[bt])
                    nc.sync.dma_start(
                        out=y_ext[bt * P : (bt + 1) * P, ot * OT : (ot + 1) * OT],
                        in_=o_sb,
                    )

    nc.compile()
    _PROGRAM_CACHE[repeat] = nc
    return nc


def host_prep(x, cheby_coeffs):
    """Host-side layout prep: coeffs -> [j, i, o] fp16, x row shards."""
    cj = np.ascontiguousarray(np.transpose(np.asarray(cheby_coeffs), (2, 0, 1))).astype(
        np.float16
    )
    x = np.asarray(x, dtype=np.float32).reshape(B_FULL, I_DIM)
    in_maps = [
        {"xs": x[i * B_SH : (i + 1) * B_SH], "cj": cj} for i in range(NCORES)
    ]
    return in_maps


def kernel(x, cheby_coeffs):
    nc = build_program(1)
    in_maps = host_prep(x, cheby_coeffs)
    res = run_bass_kernel_spmd(nc, in_maps, list(range(NCORES)))
    return np.concatenate([r["y"] for r in res.results], axis=0)


# revision 21
# speedup vs baseline: 1.0548x; 1.0410x over previous
"""ChebyshevKAN layer on 8 Trainium2 NeuronCores.

y[b,o] = sum_{i,j} T_j(xn[b,i]) * C[i,o,j],  xn = per-row min/max normalize to [-1,1]

Strategy (per core, batch-sharded 8 ways => 1024 rows/core):
  - normalize x rows on ACT (scale/bias per partition), cast fp16
  - DMA-transpose xn into [i, b] layout tiles
  - Chebyshev recurrence T_{j+1} = 2 xn T_j - T_{j-1} on DVE (fp16, fused
    scalar_tensor_tensor), all T_j tiles cached in SBUF (16 MB)
  - coeffs pre-transposed on host to [j, i, o] fp16; streamed twice (once per
    512-wide output tile); 1152 accumulating fp16 matmuls into 8 PSUM banks
"""

import sys

sys.path.insert(0, "/opt/trn_rl_repo")

import numpy as np

import concourse.bass as bass  # noqa: F401  (bass must import before tile)
import concourse.tile as tile
from concourse import bacc, mybir
from concourse.bass_utils import run_bass_kernel_spmd

NCORES = 8
B_FULL = 8192
B_SH = B_FULL // NCORES  # 1024 rows per core
I_DIM = 1024
O_DIM = 1024
NJ = 9  # degrees 0..8
P = 128
NBT = B_SH // P  # 8 batch tiles per core
NIC = I_DIM // P  # 8 contraction chunks
OT = 512  # output tile width
NOT = O_DIM // OT  # 2

_PROGRAM_CACHE = {}


def build_program(repeat=1):
    """Build + compile the per-core Bass program (cached).

    repeat>1 wraps the whole body in an on-device loop — used only for
    timing (amortizes host dispatch overhead over `repeat` kernel runs).
    """
    if repeat in _PROGRAM_CACHE:
        return _PROGRAM_CACHE[repeat]

    f16 = mybir.dt.float16
    f32 = mybir.dt.float32

    nc = bacc.Bacc("TRN2", target_bir_lowering=False, debug=False, num_devices=NCORES)
    xs_ext = nc.dram_tensor("xs", [B_SH, I_DIM], f32, kind="ExternalInput")
    cj_ext = nc.dram_tensor("cj", [NJ, I_DIM, O_DIM], f16, kind="ExternalInput")
    y_ext = nc.dram_tensor("y", [B_SH, O_DIM], f32, kind="ExternalOutput")

    import contextlib

    with tile.TileContext(nc) as tc:
        with (
            tc.tile_pool(name="tall", bufs=1) as tp,
            tc.tile_pool(name="xp", bufs=2) as xpool,
            tc.tile_pool(name="sm", bufs=16) as spool,
            tc.tile_pool(name="cp", bufs=3) as cpool,
            tc.tile_pool(name="op", bufs=4) as opool,
            tc.tile_pool(name="ps", bufs=8, space="PSUM") as pspool,
            tc.For_i(0, repeat, 1) if repeat > 1 else contextlib.nullcontext(),
        ):
            # T_all[:, j-1, ic, bt, :] holds T_j in transposed [i, b] layout
            T_all = tp.tile([P, NJ - 1, NIC, NBT, P], f16)
            ones1 = tp.tile([P, 1], f16)
            nc.vector.memset(ones1, 1.0)

            # ---- Phase A: normalize + transpose, per batch tile ----
            for bt in range(NBT):
                x_sb = xpool.tile([P, I_DIM], f32)
                nc.sync.dma_start(out=x_sb, in_=xs_ext[bt * P : (bt + 1) * P, :])
                mx = spool.tile([P, 1], f32)
                mn = spool.tile([P, 1], f32)
                nc.vector.tensor_reduce(
                    out=mx, in_=x_sb, op=mybir.AluOpType.max, axis=mybir.AxisListType.X
                )
                nc.vector.tensor_reduce(
                    out=mn, in_=x_sb, op=mybir.AluOpType.min, axis=mybir.AxisListType.X
                )
                st2 = spool.tile([P, 2], f32)
                s = st2[:, 0:1]
                t = st2[:, 1:2]
                rng = spool.tile([P, 1], f32)
                nc.vector.tensor_sub(out=rng, in0=mx, in1=mn)
                nc.vector.reciprocal(out=s, in_=rng)
                nc.vector.tensor_scalar_mul(s, s, 2.0)
                # t = (mn * -1) * s - 1
                nc.vector.scalar_tensor_tensor(
                    out=t, in0=mn, scalar=-1.0, in1=s,
                    op0=mybir.AluOpType.mult, op1=mybir.AluOpType.mult,
                )
                nc.vector.tensor_scalar_add(t, t, -1.0)

                xt16 = xpool.tile([P, I_DIM], f16)
                nc.scalar.activation(
                    out=xt16, in_=x_sb,
                    func=mybir.ActivationFunctionType.Identity,
                    bias=t, scale=s,
                )
                # T_1 = xn, transposed into [i, b] tiles
                nc.sync.dma_start_transpose(out=T_all[:, 0, :, bt, :], in_=xt16)

            # ---- Phase B: Chebyshev recurrence (j-major so PE can follow) ----
            for j in range(2, NJ):
                for bt in range(NBT):
                    t1 = T_all[:, 0, :, bt, :]
                    tprev = T_all[:, j - 2, :, bt, :]
                    cur = T_all[:, j - 1, :, bt, :]
                    prod = xpool.tile([P, NIC, P], f16)
                    nc.vector.scalar_tensor_tensor(
                        out=prod, in0=tprev, scalar=2.0, in1=t1,
                        op0=mybir.AluOpType.mult, op1=mybir.AluOpType.mult,
                    )
                    if j == 2:
                        nc.vector.tensor_scalar_add(cur, prod, -1.0)
                    else:
                        nc.vector.tensor_sub(
                            out=cur, in0=prod, in1=T_all[:, j - 3, :, bt, :]
                        )

            # ---- Phase C: matmuls, coeffs streamed once per output tile ----
            # T_0 = 1, so its contribution is the per-column bias
            # bias[o] = sum_i C[i,o,0]: 8 tiny M=1 matmuls per output tile,
            # then seeded into each accumulator by a K=1 rank-1 matmul --
            # replaces 128 full-size degree-0 matmuls.
            ones_row = tp.tile([1, P], f16)
            nc.vector.memset(ones_row, 1.0)
            bias16s = []
            for ot in range(NOT):
                ps_bias = pspool.tile([P, OT], f32, name=f"psb{ot}", tag="psacc")
                c0_sb = cpool.tile([P, NIC, OT], f16, name=f"c0_{ot}", tag="c_sb")
                nc.sync.dma_start(
                    out=c0_sb,
                    in_=cj_ext[0, :, ot * OT : (ot + 1) * OT].rearrange(
                        "(ic p) o -> p ic o", p=P
                    ),
                )
                for ic in range(NIC):
                    nc.tensor.matmul(
                        ps_bias[0:1, :], lhsT=ones1, rhs=c0_sb[:, ic, :],
                        start=(ic == 0), stop=(ic == NIC - 1),
                    )
                bias16 = tp.tile([1, OT], f16, name=f"bias16_{ot}", tag=f"bias16_{ot}")
                nc.scalar.copy(out=bias16, in_=ps_bias[0:1, :])
                bias16s.append(bias16)

            for ot in range(NOT):
                psums = [
                    pspool.tile([P, OT], f32, name=f"ps{ot}_{bt}", tag="psacc")
                    for bt in range(NBT)
                ]
                # seed accumulators with the T_0 bias via K=1 rank-1 matmul
                for bt in range(NBT):
                    nc.tensor.matmul(
                        psums[bt], lhsT=ones_row, rhs=bias16s[ot],
                        start=True, stop=False,
                    )
                for j in range(1, NJ):
                    c_sb = cpool.tile([P, NIC, OT], f16, name=f"c_{ot}_{j}", tag="c_sb")
                    eng = nc.sync if j % 2 == 0 else nc.scalar
                    eng.dma_start(
                        out=c_sb,
                        in_=cj_ext[j, :, ot * OT : (ot + 1) * OT].rearrange(
                            "(ic p) o -> p ic o", p=P
                        ),
                    )
                    for ic in range(NIC):
                        for bt in range(NBT):
                            nc.tensor.matmul(
                                psums[bt], lhsT=T_all[:, j - 1, ic, bt, :], rhs=c_sb[:, ic, :],
                                start=False,
                                stop=(j == NJ - 1 and ic == NIC - 1),
                            )
                for bt in range(NBT):
                    o_sb = opool.tile([P, OT], f32)
                    nc.scalar.copy(out=o_sb, in_=psums[bt])
                    nc.sync.dma_start(
                        out=y_ext[bt * P : (bt + 1) * P, ot * OT : (ot + 1) * OT],
                        in_=o_sb,
                    )

    nc.compile()
    _PROGRAM_CACHE[repeat] = nc
    return nc


def host_prep(x, cheby_coeffs):
    """Host-side layout prep: coeffs -> [j, i, o] fp16, x row shards."""
    cj = np.ascontiguousarray(np.transpose(np.asarray(cheby_coeffs), (2, 0, 1))).astype(
        np.float16
    )
    x = np.asarray(x, dtype=np.float32).reshape(B_FULL, I_DIM)
    in_maps = [
        {"xs": x[i * B_SH : (i + 1) * B_SH], "cj": cj} for i in range(NCORES)
    ]
    return in_maps


def kernel(x, cheby_coeffs):
    nc = build_program(1)
    in_maps = host_prep(x, cheby_coeffs)
    res = run_bass_kernel_spmd(nc, in_maps, list(range(NCORES)))
    return np.concatenate([r["y"] for r in res.results], axis=0)


# revision 22
# speedup vs baseline: 1.0957x; 1.0387x over previous
"""ChebyshevKAN layer on 8 Trainium2 NeuronCores.

y[b,o] = sum_{i,j} T_j(xn[b,i]) * C[i,o,j],  xn = per-row min/max normalize to [-1,1]

Strategy (per core, batch-sharded 8 ways => 1024 rows/core):
  - normalize x rows on ACT (scale/bias per partition), cast fp16
  - DMA-transpose xn into [i, b] layout tiles
  - Chebyshev recurrence T_{j+1} = 2 xn T_j - T_{j-1} on DVE (fp16, fused
    scalar_tensor_tensor), all T_j tiles cached in SBUF (16 MB)
  - coeffs pre-transposed on host to [j, i, o] fp16; streamed twice (once per
    512-wide output tile); 1152 accumulating fp16 matmuls into 8 PSUM banks
"""

import sys

sys.path.insert(0, "/opt/trn_rl_repo")

import numpy as np

import concourse.bass as bass  # noqa: F401  (bass must import before tile)
import concourse.tile as tile
from concourse import bacc, mybir
from concourse.bass_utils import run_bass_kernel_spmd

NCORES = 8
B_FULL = 8192
B_SH = B_FULL // NCORES  # 1024 rows per core
I_DIM = 1024
O_DIM = 1024
NJ = 9  # degrees 0..8
P = 128
NBT = B_SH // P  # 8 batch tiles per core
NIC = I_DIM // P  # 8 contraction chunks
OT = 512  # output tile width
NOT = O_DIM // OT  # 2

_PROGRAM_CACHE = {}


def build_program(repeat=1):
    """Build + compile the per-core Bass program (cached).

    repeat>1 wraps the whole body in an on-device loop — used only for
    timing (amortizes host dispatch overhead over `repeat` kernel runs).
    """
    if repeat in _PROGRAM_CACHE:
        return _PROGRAM_CACHE[repeat]

    f16 = mybir.dt.float16
    f32 = mybir.dt.float32

    nc = bacc.Bacc("TRN2", target_bir_lowering=False, debug=False, num_devices=NCORES)
    xs_ext = nc.dram_tensor("xs", [B_SH, I_DIM], f32, kind="ExternalInput")
    cj_ext = nc.dram_tensor("cj", [NJ, I_DIM, O_DIM], f16, kind="ExternalInput")
    y_ext = nc.dram_tensor("y", [B_SH, O_DIM], f32, kind="ExternalOutput")

    import contextlib

    with tile.TileContext(nc) as tc:
        with (
            tc.tile_pool(name="tall", bufs=1) as tp,
            tc.tile_pool(name="xp", bufs=2) as xpool,
            tc.tile_pool(name="sm", bufs=16) as spool,
            tc.tile_pool(name="cp", bufs=3) as cpool,
            tc.tile_pool(name="op", bufs=4) as opool,
            tc.tile_pool(name="ps", bufs=8, space="PSUM") as pspool,
            tc.For_i(0, repeat, 1) if repeat > 1 else contextlib.nullcontext(),
        ):
            # T_all[:, j-1, ic, bt, :] holds T_j in transposed [i, b] layout
            T_all = tp.tile([P, NJ - 1, NIC, NBT, P], f16)
            ones1 = tp.tile([P, 1], f16)
            nc.vector.memset(ones1, 1.0)

            # ---- Phase A: normalize + transpose, per batch tile ----
            for bt in range(NBT):
                x_sb = xpool.tile([P, I_DIM], f32)
                nc.sync.dma_start(out=x_sb, in_=xs_ext[bt * P : (bt + 1) * P, :])
                mx = spool.tile([P, 1], f32)
                mn = spool.tile([P, 1], f32)
                nc.vector.tensor_reduce(
                    out=mx, in_=x_sb, op=mybir.AluOpType.max, axis=mybir.AxisListType.X
                )
                nc.vector.tensor_reduce(
                    out=mn, in_=x_sb, op=mybir.AluOpType.min, axis=mybir.AxisListType.X
                )
                st2 = spool.tile([P, 2], f32)
                s = st2[:, 0:1]
                t = st2[:, 1:2]
                rng = spool.tile([P, 1], f32)
                nc.vector.tensor_sub(out=rng, in0=mx, in1=mn)
                nc.vector.reciprocal(out=s, in_=rng)
                nc.vector.tensor_scalar_mul(s, s, 2.0)
                # t = (mn * -1) * s - 1
                nc.vector.scalar_tensor_tensor(
                    out=t, in0=mn, scalar=-1.0, in1=s,
                    op0=mybir.AluOpType.mult, op1=mybir.AluOpType.mult,
                )
                nc.vector.tensor_scalar_add(t, t, -1.0)

                xt16 = xpool.tile([P, I_DIM], f16)
                nc.scalar.activation(
                    out=xt16, in_=x_sb,
                    func=mybir.ActivationFunctionType.Identity,
                    bias=t, scale=s,
                )
                # T_1 = xn, transposed into [i, b] tiles
                nc.sync.dma_start_transpose(out=T_all[:, 0, :, bt, :], in_=xt16)

            # ---- Phase B: Chebyshev recurrence (j-major so PE can follow) ----
            for j in range(2, NJ):
                for bt in range(NBT):
                    t1 = T_all[:, 0, :, bt, :]
                    tprev = T_all[:, j - 2, :, bt, :]
                    cur = T_all[:, j - 1, :, bt, :]
                    prod = xpool.tile([P, NIC, P], f16)
                    nc.vector.scalar_tensor_tensor(
                        out=prod, in0=tprev, scalar=2.0, in1=t1,
                        op0=mybir.AluOpType.mult, op1=mybir.AluOpType.mult,
                    )
                    if j == 2:
                        nc.vector.tensor_scalar_add(cur, prod, -1.0)
                    else:
                        nc.vector.tensor_sub(
                            out=cur, in0=prod, in1=T_all[:, j - 3, :, bt, :]
                        )

            # ---- Phase C: matmuls, coeffs streamed once per output tile ----
            # T_0 = 1, so its contribution is the per-column bias
            # bias[o] = sum_i C[i,o,0]: 8 tiny M=1 matmuls per output tile,
            # then seeded into each accumulator by a K=1 rank-1 matmul --
            # replaces 128 full-size degree-0 matmuls.
            ones_row = tp.tile([1, P], f16)
            nc.vector.memset(ones_row, 1.0)
            bias16s = []
            for ot in range(NOT):
                ps_bias = pspool.tile([P, OT], f32, name=f"psb{ot}", tag="psacc")
                c0_sb = cpool.tile([P, NIC, OT], f16, name=f"c0_{ot}", tag="c_sb")
                nc.sync.dma_start(
                    out=c0_sb,
                    in_=cj_ext[0, :, ot * OT : (ot + 1) * OT].rearrange(
                        "(ic p) o -> p ic o", p=P
                    ),
                )
                for ic in range(NIC):
                    nc.tensor.matmul(
                        ps_bias[0:1, :], lhsT=ones1, rhs=c0_sb[:, ic, :],
                        start=(ic == 0), stop=(ic == NIC - 1),
                    )
                bias16 = tp.tile([1, OT], f16, name=f"bias16_{ot}", tag=f"bias16_{ot}")
                nc.scalar.copy(out=bias16, in_=ps_bias[0:1, :])
                bias16s.append(bias16)

            for ot in range(NOT):
                psums = [
                    pspool.tile([P, OT], f32, name=f"ps{ot}_{bt}", tag="psacc")
                    for bt in range(NBT)
                ]
                # seed accumulators with the T_0 bias via K=1 rank-1 matmul
                for bt in range(NBT):
                    nc.tensor.matmul(
                        psums[bt], lhsT=ones_row, rhs=bias16s[ot],
                        start=True, stop=False,
                    )
                for j in range(1, NJ):
                    c_sb = cpool.tile([P, NIC, OT], f16, name=f"c_{ot}_{j}", tag="c_sb")
                    # split each slab across both HWDGE issuing engines:
                    # halves per-slab arrival latency vs alternating whole slabs
                    half = cj_ext[j, :, ot * OT : (ot + 1) * OT].rearrange(
                        "(ic p) o -> p ic o", p=P
                    )
                    nc.sync.dma_start(out=c_sb[:, 0 : NIC // 2, :], in_=half[:, 0 : NIC // 2, :])
                    nc.scalar.dma_start(out=c_sb[:, NIC // 2 :, :], in_=half[:, NIC // 2 :, :])
                    for ic in range(NIC):
                        for bt in range(NBT):
                            nc.tensor.matmul(
                                psums[bt], lhsT=T_all[:, j - 1, ic, bt, :], rhs=c_sb[:, ic, :],
                                start=False,
                                stop=(j == NJ - 1 and ic == NIC - 1),
                            )
                for bt in range(NBT):
                    o_sb = opool.tile([P, OT], f32)
                    nc.scalar.copy(out=o_sb, in_=psums[bt])
                    nc.sync.dma_start(
                        out=y_ext[bt * P : (bt + 1) * P, ot * OT : (ot + 1) * OT],
                        in_=o_sb,
                    )

    nc.compile()
    _PROGRAM_CACHE[repeat] = nc
    return nc


def host_prep(x, cheby_coeffs):
    """Host-side layout prep: coeffs -> [j, i, o] fp16, x row shards."""
    cj = np.ascontiguousarray(np.transpose(np.asarray(cheby_coeffs), (2, 0, 1))).astype(
        np.float16
    )
    x = np.asarray(x, dtype=np.float32).reshape(B_FULL, I_DIM)
    in_maps = [
        {"xs": x[i * B_SH : (i + 1) * B_SH], "cj": cj} for i in range(NCORES)
    ]
    return in_maps


def kernel(x, cheby_coeffs):
    nc = build_program(1)
    in_maps = host_prep(x, cheby_coeffs)
    res = run_bass_kernel_spmd(nc, in_maps, list(range(NCORES)))
    return np.concatenate([r["y"] for r in res.results], axis=0)
